# revision 24
# baseline (speedup 1.0000x reference)
"""FP8 fake-quant matmul on 8 TRN2 NeuronCores.

Computes reference semantics:
    w_dq = fq(weight, s_w);  x_dq = fq(x.reshape(-1,K), s_x)
    out  = (x_dq @ w_dq).reshape(B, S, N)
where fq(t, s) = clip(t*s, +-448) round-tripped through float8_e4m3fn (OCP),
s = 448 / amax(|t|).

Device strategy (data-parallel over rows M = B*S, 8 shards, one per core):
  The GEMM is the only device-roofline work here: 2048 DoubleRow fp8 matmuls
  per core at the PE's measured fp8 peak (512 cols x ~0.42 ns = 216 ns each,
  LDWEIGHTS fully overlapped, 512-col moving is the ISA cap) = ~440 us.
  Everything else is host prep:

  Host: amax + scales in exact fp32 (matches reference arithmetic), then
  quantizes both tensors to TRN fp8e4 at HALF the reference scale -- TRN
  fp8e4 (IEEE e4m3) tops out at 240 vs OCP e4m3fn's 448, and |t|*s/2 <= 224
  rounds identically to OCP at full scale (exponent shift), so the round-trip
  bits match the reference except for a ~1e-4-fraction subnormal tail.
  Operands are pre-packed so every DMA piece is one fully-linear block per
  partition (4 KiB lines), batched 4 chunks at a time -- DMA triggers cost
  ~600ns each on the Sync engine, so few big DMAs keep the trigger rate off
  the ramp's critical path:
    xq[((s*128 + p)*16 + c)*2 + i, m]   strip-major, piece = 4 chunks
    wq[((j*128 + p)*16 + c)*2 + i, n]   group-major, piece = 4 chunks
  Both use the same k = c*256 + p*2 + i DoubleRow pairing so contraction
  aligns.

  Single launch per core: DMA fp8 -> 2048 chunk-major DoubleRow matmuls
  (tiles of (j-group of 512 N-cols) x (strip of 4 M-tiles), 8 psum banks,
  emitted in data-arrival order) -> DVE dequant (x 4/(s_x*s_w)) -> fp32 out
  (one DMA per (j, strip)).
"""

import sys

for _p in ("/opt/trn_rl_repo", "/root/.axon_site"):
    if _p not in sys.path:
        sys.path.insert(0, _p)

import ml_dtypes
import numpy as np

import concourse.bass as bass  # noqa: F401  (registers engine classes)
import concourse.tile as tile
from concourse import bacc, mybir
from concourse.bass_utils import run_bass_kernel_spmd

# Problem shapes (hardcoded per spec)
B, S, K, N = 8, 2048, 4096, 4096
NCORES = 8
MS = (B * S) // NCORES  # 2048 rows of x per core
P = 128
FP32 = mybir.dt.float32
FP8 = mybir.dt.float8e4
NP_FP8 = ml_dtypes.float8_e4m3
FP8_MAX = np.float32(448.0)

MT = MS // P  # 16 m-tiles
CT = (K // P) // 2  # 16 DoubleRow chunks of 256 contraction rows
NB = 512  # psum bank width (fp32)
NT = N // NB  # 8 column groups
ST = 4  # m-strips of 4 m-tiles (512 cols of x^T)
SB = MS // ST
DR = mybir.MatmulPerfMode.DoubleRow

_CACHE = {}


def _build_main():
    """Single launch: pre-quantized fp8 DoubleRow matmul + dequant."""
    nc = bacc.Bacc(None, target_bir_lowering=False, debug=False)
    xq = nc.declare_dram_parameter("xq", [ST * P * CT * 2, SB], FP8, isOutput=False)
    wq = nc.declare_dram_parameter("wq", [NT * P * CT * 2, NB], FP8, isOutput=False)
    sc = nc.declare_dram_parameter("sc", [1, 8], FP32, isOutput=False)
    out = nc.declare_dram_parameter("out", [MS, N], FP32, isOutput=True)
    with tile.TileContext(nc) as tc:
        with (
            tc.tile_pool(name="const", bufs=1) as cst,
            tc.tile_pool(name="xq", bufs=ST) as xqp,
            tc.tile_pool(name="wq", bufs=4) as wqp,
            tc.tile_pool(name="ob", bufs=4) as obp,
            tc.tile_pool(name="mps", bufs=8, space="PSUM") as mpsp,
        ):
            # k <-> (p, i) pairing: k = c*256 + p*2 + i for both operands;
            # pieces of 4 chunks are one linear 4 KiB block per partition.
            xv = xq[:].rearrange("(s p c i) m -> s p c i m", p=P, c=CT, i=2)
            wv = wq[:].rearrange("(j p c i) n -> j p c i n", p=P, c=CT, i=2)
            op = out[:].rearrange("(t p) n -> p t n", p=P)

            xs = [
                xqp.tile([P, CT, 2, SB], FP8, tag="xq", name=f"xs_{s}")
                for s in range(ST)
            ]

            def emit_xpiece(s, c0, c1):
                nc.sync.dma_start(
                    out=xs[s][:, c0:c1, :, :], in_=xv[s][:, c0:c1, :, :]
                )

            wtiles = {}

            def emit_wpiece(j, c0, c1):
                if j not in wtiles:
                    wtiles[j] = wqp.tile(
                        [P, CT, 2, NB], FP8, tag="wq", name=f"wt_{j}"
                    )
                nc.sync.dma_start(
                    out=wtiles[j][:, c0:c1, :, :], in_=wv[j][:, c0:c1, :, :]
                )

            def emit_wgroup(j):
                emit_wpiece(j, 0, 8)
                emit_wpiece(j, 8, CT)

            scs = None

            def mm_tile(*tiles, drain=False):
                # (j-group, m-strip) tiles fused chunk-major: each 256-row
                # chunk is consumed for every listed tile the moment it lands.
                # One tile = 4 psum banks, so at most 2 tiles per call.
                # drain=True runs m-major instead so each psum is evacuated
                # while the next accumulates (for the final tile's tail).
                def stationary(c, m):
                    return xs[m // 4][:, c, :, (m % 4) * P : (m % 4 + 1) * P]

                def moving(j, c):
                    return wtiles[j][:, c, :, :]

                psums = {}
                for j, s in tiles:
                    for m in range(4 * s, 4 * s + 4):
                        psums[(j, m)] = mpsp.tile(
                            [P, NB], FP32, tag="mps", name=f"mps_{j}_{m}"
                        )

                def evac(j, s):
                    ob = obp.tile([P, 4, NB], FP32, tag="ob", name=f"ob_{j}_{s}")
                    for m in range(4 * s, 4 * s + 4):
                        nc.vector.tensor_scalar_mul(
                            ob[:, m % 4, :], psums[(j, m)][:], scs[:, 0:1]
                        )
                    nc.sync.dma_start(
                        out=op[:, 4 * s : 4 * s + 4, j * NB : (j + 1) * NB],
                        in_=ob[:],
                    )

                if drain:
                    # m-major with per-m evac + DMA so only one evac chain
                    # trails the final matmul (the batched (j,s) evac would
                    # leave ~6us of DVE+DMA dangling past the last MM).
                    for j, s in tiles:
                        for m in range(4 * s, 4 * s + 4):
                            for c in range(CT):
                                nc.tensor.matmul(
                                    psums[(j, m)][:],
                                    stationary(c, m),
                                    moving(j, c),
                                    start=(c == 0),
                                    stop=(c == CT - 1),
                                    perf_mode=DR,
                                )
                            ob = obp.tile(
                                [P, 4, NB], FP32, tag="ob", name=f"obd_{j}_{m}"
                            )
                            if m == 4 * s + 3:
                                # Final tile: halved evac so the exit
                                # barrier's DMA wait clears sooner.
                                HB = NB // 2
                                for h in range(2):
                                    nc.vector.tensor_scalar_mul(
                                        ob[:, 0, h * HB : (h + 1) * HB],
                                        psums[(j, m)][:, h * HB : (h + 1) * HB],
                                        scs[:, 0:1],
                                    )
                                    nc.sync.dma_start(
                                        out=op[
                                            :,
                                            m,
                                            j * NB + h * HB : j * NB
                                            + (h + 1) * HB,
                                        ],
                                        in_=ob[:, 0, h * HB : (h + 1) * HB],
                                    )
                            else:
                                nc.vector.tensor_scalar_mul(
                                    ob[:, 0, :], psums[(j, m)][:], scs[:, 0:1]
                                )
                                nc.sync.dma_start(
                                    out=op[:, m, j * NB : (j + 1) * NB],
                                    in_=ob[:, 0, :],
                                )
                    return
                for c in range(CT):
                    for j, s in tiles:
                        for m in range(4 * s, 4 * s + 4):
                            nc.tensor.matmul(
                                psums[(j, m)][:],
                                stationary(c, m),
                                moving(j, c),
                                start=(c == 0),
                                stop=(c == CT - 1),
                                perf_mode=DR,
                            )
                for j, s in tiles:
                    evac(j, s)

            # DMA emission order == data-arrival order; the PE executes
            # matmuls in emission order, so tiles are placed so the work
            # unlocked by each DMA group always exceeds what the PE can have
            # consumed when it lands.  The first pieces are 2 chunks so the
            # first matmul can issue as early as possible after the ~10us of
            # framework preamble + cold-DMA latency.  mm_tiles are
            # interleaved with the emission loops so the wq pool (4 w-groups)
            # always sees its readers before reuse.
            # The first group is a single tile: (0,0) completes on just
            # x-strip0 + w0 (4 MiB) vs 8 MiB for a fused pair -- the early
            # phase is DMA-bandwidth-bound, so halving the first group's
            # data keeps the PE fed from the first chunk.  w1 streams while
            # (0,0) executes.
            # The DMA trigger ring is 8 deep (8 semaphores round-robin):
            # trigger N+8 blocks until DMA N completes, so L1 must stay
            # within 7 triggers or the stream serializes against completions
            # right when the PE is hungriest.  Pieces: small first pair for
            # the earliest possible first matmul, then two big pieces per
            # tensor; sc rides the last slot.
            for c0, c1 in ((0, 2), (2, 8), (8, CT)):
                emit_xpiece(0, c0, c1)
                emit_wpiece(0, c0, c1)
            scs = cst.tile([P, 8], FP32)
            nc.sync.dma_start(out=scs[:], in_=sc[:].to_broadcast([P, 8]))
            emit_wgroup(1)
            mm_tile((0, 0))
            mm_tile((1, 0))
            emit_xpiece(1, 0, 8)
            emit_wpiece(2, 0, 8)
            emit_xpiece(1, 8, CT)
            emit_wpiece(2, 8, CT)
            mm_tile((0, 1), (1, 1))
            mm_tile((2, 0))
            emit_xpiece(2, 0, 8)
            emit_wpiece(3, 0, 8)
            emit_xpiece(2, 8, CT)
            emit_wpiece(3, 8, CT)
            mm_tile((2, 1))
            mm_tile((0, 2))
            mm_tile((1, 2))
            mm_tile((2, 2))
            mm_tile((3, 0))
            mm_tile((3, 1))
            mm_tile((3, 2))
            emit_xpiece(3, 0, 8)
            emit_xpiece(3, 8, CT)
            mm_tile((0, 3))
            mm_tile((1, 3))
            mm_tile((2, 3))
            mm_tile((3, 3))
            for j in range(4, NT):
                emit_wgroup(j)
                for s in range(ST):
                    mm_tile((j, s), drain=(j == NT - 1 and s == ST - 1))
    nc.compile()
    return nc


def _get(name, builder):
    if name not in _CACHE:
        _CACHE[name] = builder()
    return _CACHE[name]


def _prepare(x: np.ndarray, weight: np.ndarray):
    """Host prep: exact-fp32 scales, fp8 quantization, DMA-friendly packing.

    Returns (in_maps, core_ids).
    """
    x = np.asarray(x, dtype=np.float32)
    weight = np.asarray(weight, dtype=np.float32)
    assert x.shape == (B, S, K) and weight.shape == (K, N)
    x2d = x.reshape(B * S, K)

    # Exact reference scale arithmetic (fp32 throughout).  Quantization runs
    # at HALF the reference scale (TRN fp8e4 max-normal 240 vs OCP 448);
    # |t|*s/2 <= 224 needs no clip and rounds identically to OCP.
    amax_x = np.float32(max(x2d.max(initial=0.0), -x2d.min(initial=0.0)))
    amax_w = np.float32(max(weight.max(initial=0.0), -weight.min(initial=0.0)))
    s_x = FP8_MAX / np.maximum(amax_x, np.float32(1e-12))
    s_w = FP8_MAX / np.maximum(amax_w, np.float32(1e-12))
    dq = np.float32(4.0) * (np.float32(1.0) / s_x) * (np.float32(1.0) / s_w)
    scales = np.zeros((1, 8), np.float32)
    scales[0, 0] = dq

    qx = (x2d * (s_x * np.float32(0.5))).astype(NP_FP8)  # [M, K]
    qw = (weight * (s_w * np.float32(0.5))).astype(NP_FP8)  # [K, N]

    # wq packed [j, p, c, i, n]: k = c*256 + p*2 + i, n-group j.
    wq_packed = np.ascontiguousarray(
        qw.reshape(CT, P, 2, NT, NB).transpose(3, 1, 0, 2, 4)
    ).reshape(NT * P * CT * 2, NB)

    core_ids = list(range(NCORES))
    in_maps = []
    for c in core_ids:
        # xq packed [s, p, c, i, m]: same k pairing, m-strip-major.
        shard = qx[c * MS : (c + 1) * MS].T  # [K, MS] view
        xq_packed = np.ascontiguousarray(
            shard.reshape(CT, P, 2, ST, SB).transpose(3, 1, 0, 2, 4)
        ).reshape(ST * P * CT * 2, SB)
        in_maps.append({"xq": xq_packed, "wq": wq_packed, "sc": scales})
    return in_maps, core_ids


def _run(x: np.ndarray, weight: np.ndarray, trace: bool = False):
    in_maps, core_ids = _prepare(x, weight)
    nc = _get("main", _build_main)
    res = run_bass_kernel_spmd(nc, in_maps, core_ids, trace=trace)
    out = np.concatenate([res.results[c]["out"] for c in core_ids], axis=0)
    return out.reshape(B, S, N), res


def kernel(x: np.ndarray, weight: np.ndarray) -> np.ndarray:
    out, _ = _run(x, weight)
    return out
